# revision 4
# baseline (speedup 1.0000x reference)
"""CrossAttentionBlock TRN2 kernel (8 NeuronCores).

Sharding: core (b, g) = batch b in 0..3, head-group g in 0..1 (8 heads each).
Per core: Q/K/V projections for its head-group (f32r matmuls at full PE rate),
transposed-layout attention (softmax without max-subtraction; mask folded into
the fused exp as a per-partition bias; denominators via all-ones matmul),
partial output projection, pairwise ReduceScatter over (2b, 2b+1), then fused
residual + LayerNorm on the owned rows.

All matmuls run in float32r (TF32-like, ~6e-5 rel err, 1 cycle/row).
"""

import os
import sys

sys.path.insert(0, "/opt/trn_rl_repo")

import numpy as np
from contextlib import ExitStack

import concourse.bass as bass
from concourse import bacc
import concourse.mybir as mybir
import concourse.tile as tile

F32 = mybir.dt.float32
F32R = mybir.dt.float32r
AF = mybir.ActivationFunctionType
ALU = mybir.AluOpType

B, S, A, H, NH, DH = 4, 2048, 512, 2048, 16, 128
G = 2            # head groups (cores per batch)
HG = H // G      # 1024 channels per group
NHG = NH // G    # 8 heads per group
SCW = 512        # s-chunk width
NSC = S // SCW   # 4 chunks
EPS = 1e-5
SM_SCALE = float(1.0 / np.sqrt(DH))

_CACHE = {}


def _build():
    nc = bacc.Bacc("TRN2", target_bir_lowering=False, debug=False, num_devices=8)

    xt = nc.dram_tensor("xt", [H, S], F32R, kind="ExternalInput").ap()
    aut = nc.dram_tensor("aut", [H, A], F32R, kind="ExternalInput").ap()
    wq = nc.dram_tensor("wq", [H, HG], F32R, kind="ExternalInput").ap()
    wk = nc.dram_tensor("wk", [H, HG], F32R, kind="ExternalInput").ap()
    wv = nc.dram_tensor("wv", [H, HG], F32R, kind="ExternalInput").ap()
    wo = nc.dram_tensor("wo", [HG, H], F32R, kind="ExternalInput").ap()
    ones_in = nc.dram_tensor("ones", [128, 128], F32R, kind="ExternalInput").ap()
    bqT = nc.dram_tensor("bqT", [128, NHG], F32, kind="ExternalInput").ap()
    bkT = nc.dram_tensor("bkT", [128, NHG], F32, kind="ExternalInput").ap()
    maskT = nc.dram_tensor("maskT", [128, A // 128], F32, kind="ExternalInput").ap()
    resid = nc.dram_tensor("resid", [S // G, H], F32, kind="ExternalInput").ap()
    gamma_bc = nc.dram_tensor("gamma_bc", [128, H], F32, kind="ExternalInput").ap()
    beta_bc = nc.dram_tensor("beta_bc", [128, H], F32, kind="ExternalInput").ap()

    po_in = nc.dram_tensor("po_in", [S, H], F32)
    po_out = nc.dram_tensor("po_out", [S // G, H], F32)
    y = nc.dram_tensor("y", [S // G, H], F32, kind="ExternalOutput").ap()

    groups = [[0, 1], [2, 3], [4, 5], [6, 7]]
    HK = H // 128  # 16 contraction chunks
    AC = A // 128  # 4 audio chunks

    with tile.TileContext(nc) as tc:
        with ExitStack() as ctx:
            # ---------------- constants / residents ----------------
            cpool = ctx.enter_context(tc.tile_pool(name="consts", bufs=1))
            ones_sb = cpool.tile([128, 128], F32R)
            nc.sync.dma_start(ones_sb[:], ones_in)
            bq_sb = cpool.tile([128, NHG], F32)
            nc.sync.dma_start(bq_sb[:], bqT)
            bk_sb = cpool.tile([128, NHG], F32)
            nc.sync.dma_start(bk_sb[:], bkT)
            mask_sb = cpool.tile([128, AC], F32)
            nc.sync.dma_start(mask_sb[:], maskT)
            eps_sb = cpool.tile([128, 1], F32)
            nc.vector.memset(eps_sb[:], EPS)
            gamma_sb = cpool.tile([128, H], F32)
            nc.sync.dma_start(gamma_sb[:], gamma_bc)
            beta_sb = cpool.tile([128, H], F32)
            nc.sync.dma_start(beta_sb[:], beta_bc)

            kv_pool = ctx.enter_context(tc.tile_pool(name="ktv", bufs=1))
            KT = kv_pool.tile([128, NHG, A], F32R)     # [dh, head, a]
            V = kv_pool.tile([128, AC, HG], F32R)      # [a_in_chunk, a_chunk, c]

            # ---------------- phase A: K^T and V ----------------
            with ExitStack() as actx:
                apool = actx.enter_context(tc.tile_pool(name="phA", bufs=1))
                aut_sb = apool.tile([128, HK, A], F32R)
                for hk in range(HK):
                    nc.sync.dma_start(
                        aut_sb[:, hk, :], aut[hk * 128:(hk + 1) * 128, :]
                    )
                awpool = actx.enter_context(tc.tile_pool(name="phAw", bufs=3))
                apsum = actx.enter_context(
                    tc.tile_pool(name="phAp", bufs=8, space="PSUM")
                )
                with nc.named_scope("kproj"):
                    psk = []
                    for m in range(NHG):
                        pk = apsum.tile([128, A], F32, tag="pk", bufs=8)
                        psk.append(pk)
                    for hk in range(HK):
                        wk_t = awpool.tile([128, HG], F32R, tag="aw", bufs=3)
                        nc.sync.dma_start(
                            wk_t[:], wk[hk * 128:(hk + 1) * 128, :]
                        )
                        for m in range(NHG):
                            nc.tensor.matmul(
                                psk[m][:],
                                wk_t[:, m * 128:(m + 1) * 128],
                                aut_sb[:, hk, :],
                                start=(hk == 0),
                                stop=(hk == HK - 1),
                            )
                    for m in range(NHG):
                        nc.scalar.activation(
                            KT[:, m, :], psk[m][:], AF.Identity,
                            bias=bk_sb[:, m:m + 1],
                        )
                with nc.named_scope("vproj"):
                    psv = []
                    for i in range(8):
                        pv = apsum.tile([128, 512], F32, tag="pk", bufs=8)
                        psv.append(pv)
                    for hk in range(HK):
                        wv_t = awpool.tile([128, HG], F32R, tag="aw", bufs=3)
                        nc.sync.dma_start(
                            wv_t[:], wv[hk * 128:(hk + 1) * 128, :]
                        )
                        for ac in range(AC):
                            for n in range(2):
                                nc.tensor.matmul(
                                    psv[ac * 2 + n][:],
                                    aut_sb[:, hk, ac * 128:(ac + 1) * 128],
                                    wv_t[:, n * 512:(n + 1) * 512],
                                    start=(hk == 0),
                                    stop=(hk == HK - 1),
                                )
                    for ac in range(AC):
                        for n in range(2):
                            nc.scalar.copy(
                                V[:, ac, n * 512:(n + 1) * 512],
                                psv[ac * 2 + n][:],
                            )

            # ---------------- main pools ----------------
            xpool = ctx.enter_context(tc.tile_pool(name="xts", bufs=17))
            wqpool = ctx.enter_context(tc.tile_pool(name="wqs", bufs=6))
            qpool = ctx.enter_context(tc.tile_pool(name="qts", bufs=9))
            epool = ctx.enter_context(tc.tile_pool(name="eps", bufs=6))
            rpool = ctx.enter_context(tc.tile_pool(name="rcs", bufs=2))
            ctpool = ctx.enter_context(tc.tile_pool(name="cts", bufs=9))
            wopool = ctx.enter_context(tc.tile_pool(name="wos", bufs=10))
            opool = ctx.enter_context(tc.tile_pool(name="ots", bufs=3))
            lpool = ctx.enter_context(tc.tile_pool(name="lns", bufs=2))
            spool = ctx.enter_context(tc.tile_pool(name="lsc", bufs=4))
            mpsum = ctx.enter_context(tc.tile_pool(name="mps", bufs=1, space="PSUM"))

            for sc in range(NSC):
                s0 = sc * SCW
                # ---- load X^T slice ----
                xt_tiles = []
                for hk in range(HK):
                    xt_t = xpool.tile([128, SCW], F32R, tag="xt", bufs=17)
                    nc.sync.dma_start(
                        xt_t[:], xt[hk * 128:(hk + 1) * 128, s0:s0 + SCW]
                    )
                    xt_tiles.append(xt_t)
                # ---- Q^T projection (m-groups of 2 for psum) ----
                qts = []
                with nc.named_scope("qproj"):
                    for mg in range(NHG // 2):
                        pqs = []
                        for mi in range(2):
                            pq = mpsum.tile([128, SCW], F32, tag="pq", bufs=2,
                                            name=f"pq{mi}")
                            pqs.append(pq)
                        for hk in range(HK):
                            wq_t = wqpool.tile([128, 256], F32R, tag="wq", bufs=6)
                            nc.sync.dma_start(
                                wq_t[:],
                                wq[hk * 128:(hk + 1) * 128,
                                   mg * 256:(mg + 1) * 256],
                            )
                            for mi in range(2):
                                nc.tensor.matmul(
                                    pqs[mi][:],
                                    wq_t[:, mi * 128:(mi + 1) * 128],
                                    xt_tiles[hk][:],
                                    start=(hk == 0),
                                    stop=(hk == HK - 1),
                                )
                        for mi in range(2):
                            m = mg * 2 + mi
                            qt = qpool.tile([128, SCW], F32R, tag="qt", bufs=9)
                            nc.scalar.activation(
                                qt[:], pqs[mi][:], AF.Identity,
                                bias=bq_sb[:, m:m + 1],
                            )
                            qts.append(qt)
                # ---- attention per head ----
                ctxts = []
                with nc.named_scope("attn"):
                    for h in range(NHG):
                        expps = []
                        for ac in range(AC):
                            pp = mpsum.tile([128, SCW], F32, tag="pp", bufs=2)
                            nc.tensor.matmul(
                                pp[:],
                                KT[:, h, ac * 128:(ac + 1) * 128],
                                qts[h][:],
                                start=True, stop=True,
                            )
                            ep = epool.tile([128, SCW], F32R, tag="ep", bufs=6)
                            nc.scalar.activation(
                                ep[:], pp[:], AF.Exp,
                                bias=mask_sb[:, ac:ac + 1], scale=SM_SCALE,
                            )
                            expps.append(ep)
                        ps = mpsum.tile([128, SCW], F32, tag="ps", bufs=1)
                        pc = mpsum.tile([128, SCW], F32, tag="pc", bufs=1)
                        for ac in range(AC):
                            nc.tensor.matmul(
                                ps[:], ones_sb[:], expps[ac][:],
                                start=(ac == 0), stop=(ac == AC - 1),
                            )
                        for ac in range(AC):
                            nc.tensor.matmul(
                                pc[:],
                                V[:, ac, h * 128:(h + 1) * 128],
                                expps[ac][:],
                                start=(ac == 0), stop=(ac == AC - 1),
                            )
                        rc = rpool.tile([128, SCW], F32, tag="rc", bufs=2)
                        nc.vector.reciprocal(rc[:], ps[:])
                        ct = ctpool.tile([128, SCW], F32R, tag="ct", bufs=9)
                        nc.vector.tensor_mul(ct[:], pc[:], rc[:])
                        ctxts.append(ct)
                # ---- partial out-projection ----
                with nc.named_scope("outproj"):
                    for n in range(4):
                        wots = []
                        for c in range(NHG):
                            wot = wopool.tile([128, 512], F32R, tag="wo", bufs=10)
                            nc.sync.dma_start(
                                wot[:],
                                wo[c * 128:(c + 1) * 128, n * 512:(n + 1) * 512],
                            )
                            wots.append(wot)
                        for mq in range(4):
                            po = mpsum.tile([128, 512], F32, tag="po", bufs=2)
                            for c in range(NHG):
                                nc.tensor.matmul(
                                    po[:],
                                    ctxts[c][:, mq * 128:(mq + 1) * 128],
                                    wots[c][:],
                                    start=(c == 0), stop=(c == NHG - 1),
                                )
                            ot = opool.tile([128, 512], F32, tag="ot", bufs=3)
                            nc.vector.tensor_copy(ot[:], po[:])
                            nc.sync.dma_start(
                                po_in.ap()[s0 + mq * 128:s0 + mq * 128 + 128,
                                           n * 512:(n + 1) * 512],
                                ot[:],
                            )
                # ---- pairwise ReduceScatter for this chunk ----
                with nc.named_scope("rs"):
                    nc.gpsimd.collective_compute(
                        "ReduceScatter",
                        ALU.add,
                        replica_groups=groups,
                        ins=[po_in.ap()[s0:s0 + SCW, :].opt()],
                        outs=[po_out.ap()[sc * 256:(sc + 1) * 256, :].opt()],
                    )
                # ---- fused residual + LayerNorm on owned rows ----
                with nc.named_scope("ln"):
                    for j in range(2):
                        r0 = sc * 256 + j * 128
                        x_t = lpool.tile([128, H], F32, tag="x", bufs=2)
                        nc.sync.dma_start(x_t[:], po_out.ap()[r0:r0 + 128, :])
                        r_t = lpool.tile([128, H], F32, tag="r", bufs=2)
                        nc.sync.dma_start(r_t[:], resid[r0:r0 + 128, :])
                        nc.vector.tensor_add(x_t[:], x_t[:], r_t[:])
                        sum_t = spool.tile([128, 1], F32, tag="sum", bufs=4)
                        nc.vector.tensor_reduce(
                            sum_t[:], x_t[:], mybir.AxisListType.X, ALU.add
                        )
                        nmu = spool.tile([128, 1], F32, tag="nmu", bufs=4)
                        nc.scalar.mul(nmu[:], sum_t[:], -1.0 / H)
                        ssq = spool.tile([128, 1], F32, tag="ssq", bufs=4)
                        nc.scalar.activation(
                            r_t[:], x_t[:], AF.Square,
                            bias=nmu[:], accum_out=ssq[:],
                        )
                        std = spool.tile([128, 1], F32, tag="std", bufs=4)
                        nc.scalar.activation(
                            std[:], ssq[:], AF.Sqrt, scale=1.0 / H,
                            bias=eps_sb[:],
                        )
                        rstd = spool.tile([128, 1], F32, tag="rstd", bufs=4)
                        nc.vector.reciprocal(rstd[:], std[:])
                        nmr = spool.tile([128, 1], F32, tag="nmr", bufs=4)
                        nc.vector.tensor_mul(nmr[:], nmu[:], rstd[:])
                        nc.scalar.activation(
                            r_t[:], x_t[:], AF.Identity,
                            scale=rstd[:], bias=nmr[:],
                        )
                        nc.vector.tensor_mul(r_t[:], r_t[:], gamma_sb[:])
                        nc.vector.tensor_add(r_t[:], r_t[:], beta_sb[:])
                        nc.sync.dma_start(y[r0:r0 + 128, :], r_t[:])

    nc.compile()
    return nc


def _get_nc():
    if "nc" not in _CACHE:
        _CACHE["nc"] = _build()
    return _CACHE["nc"]


def _own_rows(g):
    return np.array(
        [sc * SCW + g * 256 + off for sc in range(NSC) for off in range(256)]
    )


def _prep_in_maps(hidden_states, audio_tokens, attention_mask,
                  Wq, bq, Wk, bk, Wv, bv, Wo, bo, gamma, beta):
    f = np.float32
    hs = np.asarray(hidden_states, f)
    au = np.asarray(audio_tokens, f)
    am = np.asarray(attention_mask, f)
    Wq, bq = np.asarray(Wq, f), np.asarray(bq, f)
    Wk, bk = np.asarray(Wk, f), np.asarray(bk, f)
    Wv, bv = np.asarray(Wv, f), np.asarray(bv, f)
    Wo, bo = np.asarray(Wo, f), np.asarray(bo, f)
    gamma, beta = np.asarray(gamma, f), np.asarray(beta, f)

    bo_eff = bo + bv @ Wo  # fold the V bias through the output projection
    ones = np.ones((128, 128), f)
    gamma_b = np.ascontiguousarray(np.broadcast_to(gamma, (128, H)))
    beta_b = np.ascontiguousarray(np.broadcast_to(beta, (128, H)))

    in_maps = []
    for b in range(B):
        xt = np.ascontiguousarray(hs[b].T)
        autb = np.ascontiguousarray(au[b].T)
        maskT = np.ascontiguousarray((am[b] * -10000.0).reshape(AC_, 128).T)
        for g in range(G):
            sl = slice(g * HG, (g + 1) * HG)
            rows = _own_rows(g)
            in_maps.append({
                "xt": xt,
                "aut": autb,
                "wq": np.ascontiguousarray(Wq[:, sl]),
                "wk": np.ascontiguousarray(Wk[:, sl]),
                "wv": np.ascontiguousarray(Wv[:, sl]),
                "wo": np.ascontiguousarray(Wo[sl, :]),
                "ones": ones,
                "bqT": np.ascontiguousarray(bq[sl].reshape(NHG, 128).T),
                "bkT": np.ascontiguousarray(bk[sl].reshape(NHG, 128).T),
                "maskT": maskT,
                "resid": np.ascontiguousarray(hs[b][rows] + bo_eff[None, :]),
                "gamma_bc": gamma_b,
                "beta_bc": beta_b,
            })
    return in_maps


AC_ = A // 128


def run_sharded(in_maps, trace=False):
    from concourse.bass_utils import run_bass_kernel_spmd

    nc = _get_nc()
    return run_bass_kernel_spmd(
        nc, in_maps, core_ids=list(range(8)), trace=trace,
        trace_cores=[0] if trace else None,
    )


def kernel(**inputs) -> np.ndarray:
    in_maps = _prep_in_maps(**inputs)
    trace = bool(int(os.environ.get("BASS_KERNEL_TRACE", "0")))
    r = run_sharded(in_maps, trace=trace)
    _CACHE["last_result"] = r
    out = np.empty((B, S, H), np.float32)
    for b in range(B):
        for g in range(G):
            out[b][_own_rows(g)] = r.results[b * G + g]["y"]
    return out


# revision 8
# speedup vs baseline: 1.1152x; 1.1152x over previous
"""CrossAttentionBlock TRN2 kernel (8 NeuronCores).

Sharding: core (b, g) = batch b in 0..3, head-group g in 0..1 (8 heads each).
Per core: Q/K/V projections for its head-group (f32r matmuls at full PE rate),
transposed-layout attention (softmax without max-subtraction; mask folded into
the fused exp as a per-partition bias; denominators via all-ones matmul),
partial output projection, pairwise ReduceScatter over (2b, 2b+1), then fused
residual + LayerNorm on the owned rows.

All matmuls run in float32r (TF32-like, ~6e-5 rel err, 1 cycle/row).
"""

import os
import sys

sys.path.insert(0, "/opt/trn_rl_repo")

import numpy as np
from contextlib import ExitStack

import concourse.bass as bass
from concourse import bacc
import concourse.mybir as mybir
import concourse.tile as tile

F32 = mybir.dt.float32
F32R = mybir.dt.float16  # matmul dtype (fp16: FWL fast weight loads, ~5e-4 rel)
AF = mybir.ActivationFunctionType
ALU = mybir.AluOpType

B, S, A, H, NH, DH = 4, 2048, 512, 2048, 16, 128
G = 2            # head groups (cores per batch)
HG = H // G      # 1024 channels per group
NHG = NH // G    # 8 heads per group
SCW = 512        # s-chunk width
NSC = S // SCW   # 4 chunks
EPS = 1e-5
SM_SCALE = float(1.0 / np.sqrt(DH))

_CACHE = {}


def _build():
    nc = bacc.Bacc("TRN2", target_bir_lowering=False, debug=False, num_devices=8)

    xt = nc.dram_tensor("xt", [H, S], F32R, kind="ExternalInput").ap()
    aut = nc.dram_tensor("aut", [H, A], F32R, kind="ExternalInput").ap()
    wq = nc.dram_tensor("wq", [H, HG], F32R, kind="ExternalInput").ap()
    wk = nc.dram_tensor("wk", [H, HG], F32R, kind="ExternalInput").ap()
    wv = nc.dram_tensor("wv", [H, HG], F32R, kind="ExternalInput").ap()
    wo = nc.dram_tensor("wo", [HG, H], F32R, kind="ExternalInput").ap()
    ones_in = nc.dram_tensor("ones", [128, 128], F32R, kind="ExternalInput").ap()
    bqT = nc.dram_tensor("bqT", [128, NHG], F32, kind="ExternalInput").ap()
    bkT = nc.dram_tensor("bkT", [128, NHG], F32, kind="ExternalInput").ap()
    maskT = nc.dram_tensor("maskT", [128, A // 128], F32, kind="ExternalInput").ap()
    resid = nc.dram_tensor("resid", [S // G, H], F32, kind="ExternalInput").ap()
    gamma_bc = nc.dram_tensor("gamma_bc", [128, H], F32, kind="ExternalInput").ap()
    beta_bc = nc.dram_tensor("beta_bc", [128, H], F32, kind="ExternalInput").ap()

    po_in = nc.dram_tensor("po_in", [S, H], F32)
    po_out = nc.dram_tensor("po_out", [S // G, H], F32)
    y = nc.dram_tensor("y", [S // G, H], F32, kind="ExternalOutput").ap()

    groups = [[0, 1], [2, 3], [4, 5], [6, 7]]
    HK = H // 128  # 16 contraction chunks
    AC = A // 128  # 4 audio chunks

    with tile.TileContext(nc) as tc:
        with ExitStack() as ctx:
            # ---------------- constants / residents ----------------
            cpool = ctx.enter_context(tc.tile_pool(name="consts", bufs=1))
            ones_sb = cpool.tile([128, 128], F32R)
            nc.sync.dma_start(ones_sb[:], ones_in)
            bq_sb = cpool.tile([128, NHG], F32)
            nc.sync.dma_start(bq_sb[:], bqT)
            bk_sb = cpool.tile([128, NHG], F32)
            nc.sync.dma_start(bk_sb[:], bkT)
            mask_sb = cpool.tile([128, AC], F32)
            nc.sync.dma_start(mask_sb[:], maskT)
            eps_sb = cpool.tile([128, 1], F32)
            nc.vector.memset(eps_sb[:], EPS)
            gamma_sb = cpool.tile([128, H], F32)
            nc.sync.dma_start(gamma_sb[:], gamma_bc)
            beta_sb = cpool.tile([128, H], F32)
            nc.sync.dma_start(beta_sb[:], beta_bc)

            kv_pool = ctx.enter_context(tc.tile_pool(name="ktv", bufs=1))
            KT = kv_pool.tile([128, NHG, A], F32R)     # [dh, head, a]
            V = kv_pool.tile([128, AC, HG], F32R)      # [a_in_chunk, a_chunk, c]

            # ---------------- phase A: K^T and V ----------------
            with ExitStack() as actx:
                apool = actx.enter_context(tc.tile_pool(name="phA", bufs=1))
                aut_sb = apool.tile([128, HK, A], F32R)
                for hk in range(HK):
                    nc.sync.dma_start(
                        aut_sb[:, hk, :], aut[hk * 128:(hk + 1) * 128, :]
                    )
                awpool = actx.enter_context(tc.tile_pool(name="phAw", bufs=3))
                apsum = actx.enter_context(
                    tc.tile_pool(name="phAp", bufs=8, space="PSUM")
                )
                with nc.named_scope("kproj"):
                    psk = []
                    for m in range(NHG):
                        pk = apsum.tile([128, A], F32, tag="pk", bufs=8)
                        psk.append(pk)
                    for hk in range(HK):
                        wk_t = awpool.tile([128, HG], F32R, tag="aw", bufs=3)
                        nc.sync.dma_start(
                            wk_t[:], wk[hk * 128:(hk + 1) * 128, :]
                        )
                        for m in range(NHG):
                            nc.tensor.matmul(
                                psk[m][:],
                                wk_t[:, m * 128:(m + 1) * 128],
                                aut_sb[:, hk, :],
                                start=(hk == 0),
                                stop=(hk == HK - 1),
                            )
                    for m in range(NHG):
                        nc.scalar.activation(
                            KT[:, m, :], psk[m][:], AF.Identity,
                            bias=bk_sb[:, m:m + 1],
                        )
                with nc.named_scope("vproj"):
                    psv = []
                    for i in range(8):
                        pv = apsum.tile([128, 512], F32, tag="pk", bufs=8)
                        psv.append(pv)
                    for hk in range(HK):
                        wv_t = awpool.tile([128, HG], F32R, tag="aw", bufs=3)
                        nc.sync.dma_start(
                            wv_t[:], wv[hk * 128:(hk + 1) * 128, :]
                        )
                        for ac in range(AC):
                            for n in range(2):
                                nc.tensor.matmul(
                                    psv[ac * 2 + n][:],
                                    aut_sb[:, hk, ac * 128:(ac + 1) * 128],
                                    wv_t[:, n * 512:(n + 1) * 512],
                                    start=(hk == 0),
                                    stop=(hk == HK - 1),
                                )
                    for ac in range(AC):
                        for n in range(2):
                            nc.scalar.copy(
                                V[:, ac, n * 512:(n + 1) * 512],
                                psv[ac * 2 + n][:],
                            )

            # ---------------- main pools ----------------
            xpool = ctx.enter_context(tc.tile_pool(name="xts", bufs=20))
            wqpool = ctx.enter_context(tc.tile_pool(name="wqs", bufs=12))
            qpool = ctx.enter_context(tc.tile_pool(name="qts", bufs=9))
            epool = ctx.enter_context(tc.tile_pool(name="eps", bufs=8))
            rpool = ctx.enter_context(tc.tile_pool(name="rcs", bufs=2))
            ctpool = ctx.enter_context(tc.tile_pool(name="cts", bufs=9))
            wopool = ctx.enter_context(tc.tile_pool(name="wos", bufs=12))
            opool = ctx.enter_context(tc.tile_pool(name="ots", bufs=3))
            lpool = ctx.enter_context(tc.tile_pool(name="lns", bufs=2))
            spool = ctx.enter_context(tc.tile_pool(name="lsc", bufs=4))
            mpsum = ctx.enter_context(tc.tile_pool(name="mps", bufs=1, space="PSUM"))

            for sc in range(NSC):
                s0 = sc * SCW
                # ---- load X^T slice ----
                xt_tiles = []
                for hk in range(HK):
                    xt_t = xpool.tile([128, SCW], F32R, tag="xt", bufs=20)
                    nc.sync.dma_start(
                        xt_t[:], xt[hk * 128:(hk + 1) * 128, s0:s0 + SCW]
                    )
                    xt_tiles.append(xt_t)
                # ---- Q^T projection (m-groups of 2 for psum) ----
                qts = []
                with nc.named_scope("qproj"):
                    for mg in range(NHG // 2):
                        pqs = []
                        for mi in range(2):
                            pq = mpsum.tile([128, SCW], F32, tag="pq", bufs=2,
                                            name=f"pq{mi}")
                            pqs.append(pq)
                        for hk in range(HK):
                            wq_t = wqpool.tile([128, 256], F32R, tag="wq", bufs=12)
                            nc.gpsimd.dma_start(
                                wq_t[:],
                                wq[hk * 128:(hk + 1) * 128,
                                   mg * 256:(mg + 1) * 256],
                            )
                            for mi in range(2):
                                nc.tensor.matmul(
                                    pqs[mi][:],
                                    wq_t[:, mi * 128:(mi + 1) * 128],
                                    xt_tiles[hk][:],
                                    start=(hk == 0),
                                    stop=(hk == HK - 1),
                                )
                        for mi in range(2):
                            m = mg * 2 + mi
                            qt = qpool.tile([128, SCW], F32R, tag="qt", bufs=9)
                            nc.scalar.activation(
                                qt[:], pqs[mi][:], AF.Identity,
                                bias=bq_sb[:, m:m + 1],
                            )
                            qts.append(qt)
                # ---- attention per head ----
                ctxts = []
                with nc.named_scope("attn"):
                    for h in range(NHG):
                        expps = []
                        for ac in range(AC):
                            pp = mpsum.tile([128, SCW], F32, tag="pp", bufs=2)
                            nc.tensor.matmul(
                                pp[:],
                                KT[:, h, ac * 128:(ac + 1) * 128],
                                qts[h][:],
                                start=True, stop=True,
                            )
                            ep = epool.tile([128, SCW], F32R, tag="ep", bufs=8)
                            nc.scalar.activation(
                                ep[:], pp[:], AF.Exp,
                                bias=mask_sb[:, ac:ac + 1], scale=SM_SCALE,
                            )
                            expps.append(ep)
                        ps = mpsum.tile([128, SCW], F32, tag="ps", bufs=1)
                        pc = mpsum.tile([128, SCW], F32, tag="pc", bufs=1)
                        for ac in range(AC):
                            nc.tensor.matmul(
                                ps[:], ones_sb[:], expps[ac][:],
                                start=(ac == 0), stop=(ac == AC - 1),
                            )
                        for ac in range(AC):
                            nc.tensor.matmul(
                                pc[:],
                                V[:, ac, h * 128:(h + 1) * 128],
                                expps[ac][:],
                                start=(ac == 0), stop=(ac == AC - 1),
                            )
                        rc = rpool.tile([128, SCW], F32, tag="rc", bufs=2)
                        nc.vector.reciprocal(rc[:], ps[:])
                        ct = ctpool.tile([128, SCW], F32R, tag="ct", bufs=9)
                        nc.vector.tensor_mul(ct[:], pc[:], rc[:])
                        ctxts.append(ct)
                # ---- partial out-projection ----
                with nc.named_scope("outproj"):
                    for n in range(4):
                        wots = []
                        for c in range(NHG):
                            wot = wopool.tile([128, 512], F32R, tag="wo", bufs=12)
                            nc.gpsimd.dma_start(
                                wot[:],
                                wo[c * 128:(c + 1) * 128, n * 512:(n + 1) * 512],
                            )
                            wots.append(wot)
                        for mq in range(4):
                            po = mpsum.tile([128, 512], F32, tag="po", bufs=2)
                            for c in range(NHG):
                                nc.tensor.matmul(
                                    po[:],
                                    ctxts[c][:, mq * 128:(mq + 1) * 128],
                                    wots[c][:],
                                    start=(c == 0), stop=(c == NHG - 1),
                                )
                            ot = opool.tile([128, 512], F32, tag="ot", bufs=3)
                            nc.vector.tensor_copy(ot[:], po[:])
                            nc.scalar.dma_start(
                                po_in.ap()[s0 + mq * 128:s0 + mq * 128 + 128,
                                           n * 512:(n + 1) * 512],
                                ot[:],
                            )
                # ---- pairwise ReduceScatter for this chunk ----
                with nc.named_scope("rs"):
                    nc.gpsimd.collective_compute(
                        "ReduceScatter",
                        ALU.add,
                        replica_groups=groups,
                        ins=[po_in.ap()[s0:s0 + SCW, :].opt()],
                        outs=[po_out.ap()[sc * 256:(sc + 1) * 256, :].opt()],
                    )
                # ---- fused residual + LayerNorm on owned rows ----
                with nc.named_scope("ln"):
                    for j in range(2):
                        r0 = sc * 256 + j * 128
                        x_t = lpool.tile([128, H], F32, tag="x", bufs=2)
                        nc.sync.dma_start(x_t[:], po_out.ap()[r0:r0 + 128, :])
                        r_t = lpool.tile([128, H], F32, tag="r", bufs=2)
                        nc.sync.dma_start(r_t[:], resid[r0:r0 + 128, :])
                        nc.vector.tensor_add(x_t[:], x_t[:], r_t[:])
                        sum_t = spool.tile([128, 1], F32, tag="sum", bufs=4)
                        nc.vector.tensor_reduce(
                            sum_t[:], x_t[:], mybir.AxisListType.X, ALU.add
                        )
                        nmu = spool.tile([128, 1], F32, tag="nmu", bufs=4)
                        nc.scalar.mul(nmu[:], sum_t[:], -1.0 / H)
                        ssq = spool.tile([128, 1], F32, tag="ssq", bufs=4)
                        nc.scalar.activation(
                            r_t[:], x_t[:], AF.Square,
                            bias=nmu[:], accum_out=ssq[:],
                        )
                        std = spool.tile([128, 1], F32, tag="std", bufs=4)
                        nc.scalar.activation(
                            std[:], ssq[:], AF.Sqrt, scale=1.0 / H,
                            bias=eps_sb[:],
                        )
                        rstd = spool.tile([128, 1], F32, tag="rstd", bufs=4)
                        nc.vector.reciprocal(rstd[:], std[:])
                        nmr = spool.tile([128, 1], F32, tag="nmr", bufs=4)
                        nc.vector.tensor_mul(nmr[:], nmu[:], rstd[:])
                        nc.scalar.activation(
                            r_t[:], x_t[:], AF.Identity,
                            scale=rstd[:], bias=nmr[:],
                        )
                        nc.vector.tensor_mul(r_t[:], r_t[:], gamma_sb[:])
                        nc.vector.tensor_add(r_t[:], r_t[:], beta_sb[:])
                        nc.sync.dma_start(y[r0:r0 + 128, :], r_t[:])

    nc.compile()
    return nc


def _get_nc():
    if "nc" not in _CACHE:
        _CACHE["nc"] = _build()
    return _CACHE["nc"]


def _own_rows(g):
    return np.array(
        [sc * SCW + g * 256 + off for sc in range(NSC) for off in range(256)]
    )


def _prep_in_maps(hidden_states, audio_tokens, attention_mask,
                  Wq, bq, Wk, bk, Wv, bv, Wo, bo, gamma, beta):
    f = np.float32
    hs = np.asarray(hidden_states, f)
    au = np.asarray(audio_tokens, f)
    am = np.asarray(attention_mask, f)
    Wq, bq = np.asarray(Wq, f), np.asarray(bq, f)
    Wk, bk = np.asarray(Wk, f), np.asarray(bk, f)
    Wv, bv = np.asarray(Wv, f), np.asarray(bv, f)
    Wo, bo = np.asarray(Wo, f), np.asarray(bo, f)
    gamma, beta = np.asarray(gamma, f), np.asarray(beta, f)

    bo_eff = bo + bv @ Wo  # fold the V bias through the output projection
    ones = np.ones((128, 128), f)
    gamma_b = np.ascontiguousarray(np.broadcast_to(gamma, (128, H)))
    beta_b = np.ascontiguousarray(np.broadcast_to(beta, (128, H)))

    h16 = np.float16
    in_maps = []
    for b in range(B):
        xt = np.ascontiguousarray(hs[b].T).astype(h16)
        autb = np.ascontiguousarray(au[b].T).astype(h16)
        maskT = np.ascontiguousarray((am[b] * -10000.0).reshape(AC_, 128).T)
        for g in range(G):
            sl = slice(g * HG, (g + 1) * HG)
            rows = _own_rows(g)
            in_maps.append({
                "xt": xt,
                "aut": autb,
                "wq": np.ascontiguousarray(Wq[:, sl]).astype(h16),
                "wk": np.ascontiguousarray(Wk[:, sl]).astype(h16),
                "wv": np.ascontiguousarray(Wv[:, sl]).astype(h16),
                "wo": np.ascontiguousarray(Wo[sl, :]).astype(h16),
                "ones": ones.astype(h16),
                "bqT": np.ascontiguousarray(bq[sl].reshape(NHG, 128).T),
                "bkT": np.ascontiguousarray(bk[sl].reshape(NHG, 128).T),
                "maskT": maskT,
                "resid": np.ascontiguousarray(hs[b][rows] + bo_eff[None, :]),
                "gamma_bc": gamma_b,
                "beta_bc": beta_b,
            })
    return in_maps


AC_ = A // 128


def run_sharded(in_maps, trace=False):
    from concourse.bass_utils import run_bass_kernel_spmd

    nc = _get_nc()
    return run_bass_kernel_spmd(
        nc, in_maps, core_ids=list(range(8)), trace=trace,
        trace_cores=[0] if trace else None,
    )


def kernel(**inputs) -> np.ndarray:
    in_maps = _prep_in_maps(**inputs)
    trace = bool(int(os.environ.get("BASS_KERNEL_TRACE", "0")))
    r = run_sharded(in_maps, trace=trace)
    _CACHE["last_result"] = r
    out = np.empty((B, S, H), np.float32)
    for b in range(B):
        for g in range(G):
            out[b][_own_rows(g)] = r.results[b * G + g]["y"]
    return out
